# revision 14
# baseline (speedup 1.0000x reference)
"""Bass/Tile Trainium2 kernel for the CAFBlock fusion (nn_CAFBlock).

Strategy: shard the audio channel dim C_a=128 across 8 NeuronCores (16
channels per core).  BatchNorm2d statistics are per-channel -> fully local.
The tiny video branch (gLN over all channels) is computed redundantly on
every core from a replicated copy of v1, so there are no collectives.

Per-core SBUF layout for the big tensors: partition p = b*64 + k where k is
the video-frame index (t = k*8 + r), free dim = (c_local, r, f).  With this
layout the nearest-interpolated v_attn/v_key factors are constant along the
free dim, so the fused output

    out[ns,c] = (attn_ns*Av)ated * src + key_ns * relu(Ag*src+Bg) + attn_ns*Bv

is computed on the *tensor engine* as two accumulated diagonal matmuls per
(ns, half): diag(alpha_ns) @ src + diag(key_ns) @ gate, with the rank-1
beta term folded into the PSUM->SBUF copy as a per-partition bias.  a1 is
shipped to the device in fp16 (halves input DMA traffic; rel err ~5e-4 vs
the 2e-2 gate).  BN statistics come from per-channel bn_stats pairs; all
rsqrts use an integer-Newton iteration on the DVE so the scalar engine
loads exactly one activation table (exp, for the softmax).
"""

import numpy as np

import concourse.bass as bass
import concourse.bacc as bacc
import concourse.tile as tile
import concourse.mybir as mybir
from concourse.bass_utils import run_bass_kernel_spmd

F32 = mybir.dt.float32
FP16 = mybir.dt.float16
I32 = mybir.dt.int32
AF = mybir.ActivationFunctionType
OP = mybir.AluOpType
AX = mybir.AxisListType
MS = bass.MemorySpace

# problem dims (hardcoded per the harness contract)
B, NS, CA, H, T, FQ, TV = 2, 2, 128, 4, 512, 128, 64
NCORE = 8
CL = CA // NCORE            # 16 local channels per core
N = B * NS                  # 4 (b*ns video samples)
RP = T // TV                # 8 (nearest-interp repeat factor)
BN_EPS, GLN_EPS = 1e-5, 1e-8
NBN = float(B * T * FQ)     # 131072 elements per BN channel
NKEY = float(CA * TV)       # 8192 elements per gLN(key) sample
NATT = float(CA * H * TV)   # 32768 elements per gLN(attn) sample
CF = RP * FQ                # 1024 free elements per channel tile
AFREE = CL * CF             # 16384 free elements of resident a1 shard
OFREE = CL * NS * CF        # 32768 free elements of output
GC = 8                      # channels per finalize group
K_MAGIC = 0x5F3759DF

# (c, ns) pairs whose PSUM->SBUF copy runs on the DVE instead of ACT
DVE_COPIES = frozenset(
    (c, 1) for c in (0, 2, 3, 5, 6, 8, 9, 11, 12, 14))


def _rsqrt_hack(nc, pc, q, width, pref):
    """y = 1/sqrt(q) for positive q via int bit-hack + 3 Newton steps.

    Runs entirely on the DVE (no activation tables)."""
    kcol = pc.tile([1, width], I32, tag=pref + "kc")
    t1 = pc.tile([1, width], I32, tag=pref + "t1")
    y = pc.tile([1, width], F32, tag=pref + "y")
    ysq = pc.tile([1, width], F32, tag=pref + "ys")
    nc.vector.memset(kcol[:], K_MAGIC)
    nc.vector.tensor_scalar(t1[:], q.bitcast(I32), 1, None,
                            OP.logical_shift_right)
    nc.vector.tensor_tensor(y[:].bitcast(I32), kcol[:], t1[:], OP.subtract)
    for _ in range(3):
        nc.vector.tensor_tensor(ysq[:], y[:], y[:], OP.mult)
        nc.vector.tensor_tensor(ysq[:], q, ysq[:], OP.mult)
        nc.vector.tensor_scalar(ysq[:], ysq[:], -0.5, 1.5, OP.mult, OP.add)
        nc.vector.tensor_tensor(y[:], y[:], ysq[:], OP.mult)
    return y


def _build():
    """Builds the SPMD Bass program (same program on all 8 cores)."""
    nc = bacc.Bacc("TRN2", target_bir_lowering=False, debug=False)

    d_a1 = nc.dram_tensor("a1s", [128, AFREE], FP16, kind="ExternalInput")
    # consts packed host-side:
    # cb1 [128, 266]: v1f 0:256 | pcol 256:266
    # cb2 [16, 290]:  v1l 0:256 | ploc 256:274 | id16 274:290
    # cb3 [1, 257]:   oner 0:128 | prow 128:256 | onec-col via memset
    # cbh [128, 128] fp16: identity
    d_cb1 = nc.dram_tensor("cb1", [128, 285], F32, kind="ExternalInput")
    d_cb2 = nc.dram_tensor("cb2", [CL, CL], F32, kind="ExternalInput")
    d_cb3 = nc.dram_tensor("cb3", [1, 264], F32, kind="ExternalInput")
    d_cbh = nc.dram_tensor("cbh", [128, 128], FP16, kind="ExternalInput")
    d_out = nc.dram_tensor("out", [128, OFREE], F32, kind="ExternalOutput")

    with tile.TileContext(nc) as tc:
        with (
            tc.tile_pool(name="pout", bufs=3, space=MS.PSUM) as ppo,
            tc.tile_pool(name="psmall", bufs=2, space=MS.PSUM) as pps,
            tc.tile_pool(name="pres", bufs=8) as pres,
            tc.tile_pool(name="pconst", bufs=1) as pc,
            tc.tile_pool(name="pgate", bufs=3) as pgate,
            tc.tile_pool(name="pdiag", bufs=8) as pdiag,
            tc.tile_pool(name="post", bufs=3) as post,
        ):
            # ---------------- constants (4 fast HWDGE DMAs) ---------------
            cb1 = pc.tile([128, 285], F32, tag="cb1")
            cb2 = pc.tile([CL, CL], F32, tag="cb2")
            cb3 = pc.tile([1, 264], F32, tag="cb3")
            idh = pc.tile([128, 128], FP16, tag="idh")
            res = []
            for g in range(8):
                rt = pres.tile([128, 2048], FP16, tag="res")
                res.append(rt)
            # parallel DMA issue: SP carries cb1 + first tiles, ACT the
            # remaining consts, Pool (SWDGE) the tail tiles
            nc.sync.dma_start(cb1[:], d_cb1.ap()[:])
            for g in range(0, 5):
                nc.sync.dma_start(res[g][:],
                                  d_a1.ap()[:, g * 2048:(g + 1) * 2048])
            nc.scalar.dma_start(cb3[:], d_cb3.ap()[:])
            nc.scalar.dma_start(cb2[:], d_cb2.ap()[:])
            nc.scalar.dma_start(idh[:], d_cbh.ap()[:])
            for g in range(5, 8):
                nc.gpsimd.dma_start(res[g][:],
                                    d_a1.ap()[:, g * 2048:(g + 1) * 2048])
            v1f = cb1[:, 0:256]
            wkc = cb1[:, 256:257]
            bkc = cb1[:, 257:258]
            swac = cb1[:, 258:259]
            sw2c = cb1[:, 259:260]
            swb2c = cb1[:, 260:261]
            gkx4 = cb1[:, 261:265]
            bekx4 = cb1[:, 265:269]
            wax4 = cb1[:, 269:273]
            ba4x4 = cb1[:, 273:277]
            ga4x4 = cb1[:, 277:281]
            bea4x4 = cb1[:, 281:285]
            id16 = cb2[:, 0:CL]
            oner = cb3[:, 0:128]
            prow = cb3[:, 128:256]
            sbax4 = cb3[:, 256:260]
            sba2x4 = cb3[:, 260:264]
            # prow: wv 0:16 | gv 16:32 | bev 32:48 | wg 48:64 | gg 64:80 |
            #       beg 80:96 | wv2 96:112 | wg2 112:128
            onec = pc.tile([128, 1], F32, tag="onec")
            nc.vector.memset(onec[:], 1.0)

            def a1c(c):
                return res[c // 2][:, (c % 2) * CF:(c % 2) * CF + CF]

            # ============== v-branch elementwise (DVE, latency-critical) ==
            # vkf = v1*wk + bk (per perm'd channel); squares + affine-attn
            vkf = pc.tile([128, N * TV], F32, tag="vkf")
            scrk = pc.tile([128, N * TV], F32, tag="scrk")
            uswa = pc.tile([128, N * TV], F32, tag="uswa")
            v1sq = pc.tile([128, N * TV], F32, tag="v1sq")
            za = pc.tile([128, N * TV], F32, tag="za")
            zc = pc.tile([128, N * TV], F32, tag="zc")
            zd = pc.tile([128, N * TV], F32, tag="zd")
            if True:
                nc.vector.tensor_scalar(vkf[:], v1f[:], wkc, bkc, OP.mult,
                                        OP.add)
                nc.vector.tensor_tensor(scrk[:], vkf[:], vkf[:], OP.mult)
                nc.vector.tensor_scalar(uswa[:], v1f[:], swac, None, OP.mult)
                nc.vector.tensor_tensor(v1sq[:], v1f[:], v1f[:], OP.mult)
                nc.vector.tensor_scalar(za[:], v1sq[:], sw2c, None, OP.mult)
                nc.vector.tensor_scalar(zc[:], v1f[:], swb2c, None, OP.mult)
                nc.vector.tensor_tensor(zd[:], za[:], zc[:], OP.add)

            # ============== BN stats c=0..3 (DVE) =========================
            bns = pc.tile([128, CL * 12], F32, tag="bns")

            def bn_pair(c):
                src = a1c(c)
                floor = 0.006 + 0.0012 * (c // 2)
                with tc.tile_wait_until(floor):
                    nc.vector.bn_stats(bns[:, c * 12:c * 12 + 6],
                                       src[:, 0:512])
                    nc.vector.bn_stats(bns[:, c * 12 + 6:c * 12 + 12],
                                       src[:, 512:1024])


            # ============== v-branch reductions (DVE) =====================
            red = pc.tile([128, 16], F32, tag="red")
            for (dst0, srcv) in ((0, vkf), (4, scrk), (8, uswa), (12, zd)):
                nc.vector.tensor_reduce(
                    red[:, dst0:dst0 + 4],
                    srcv[:].rearrange("p (n t) -> p n t", n=N, t=TV),
                    axis=AX.X, op=OP.add)


            bn_pair(0)
            bn_pair(1)

            # ------- v-branch cross-partition reduction + finalize --------
            pp_ks = pps.tile([128, 32], F32, tag="sm")
            nc.tensor.matmul(pp_ks[0:1, 0:16], onec[:], red[:], start=True,
                             stop=True)
            rr = pc.tile([1, 16], F32, tag="rr")
            nc.scalar.copy(rr[:], pp_ks[0:1, 0:16])
            # rr: sum_vkf 0:4 | sum_vkf2 4:8 | sum_u 8:12 | sum_z 12:16

            bn_pair(2)
            bn_pair(3)

            # b1 [1,20]: km 0:4 | rs_k 4:8 | km*rs_k 8:12 | rs_a 12:16 |
            #            am*rs_a 16:20
            b1 = pc.tile([1, 20], F32, tag="b1")
            scr = pc.tile([1, 12], F32, tag="scr")   # kE2 0:4 | am 4:8 | aE2 8:12
            qv = pc.tile([1, 8], F32, tag="qv")
            nc.vector.tensor_scalar_mul(b1[:, 0:4], rr[:, 0:4], 1.0 / NKEY)
            nc.vector.tensor_scalar_mul(scr[:, 0:4], rr[:, 4:8], 1.0 / NKEY)
            nc.vector.tensor_tensor(qv[:, 0:4], b1[:, 0:4], b1[:, 0:4],
                                    OP.mult)
            nc.vector.tensor_tensor(qv[:, 0:4], scr[:, 0:4], qv[:, 0:4],
                                    OP.subtract)
            nc.vector.tensor_tensor(scr[:, 4:8], rr[:, 8:12], sbax4, OP.add)
            nc.vector.tensor_scalar_mul(scr[:, 4:8], scr[:, 4:8], 1.0 / NATT)
            nc.vector.tensor_tensor(scr[:, 8:12], rr[:, 12:16], sba2x4,
                                    OP.add)
            nc.vector.tensor_scalar_mul(scr[:, 8:12], scr[:, 8:12],
                                        1.0 / NATT)
            nc.vector.tensor_tensor(qv[:, 4:8], scr[:, 4:8], scr[:, 4:8],
                                    OP.mult)
            nc.vector.tensor_tensor(qv[:, 4:8], scr[:, 8:12], qv[:, 4:8],
                                    OP.subtract)
            nc.vector.tensor_scalar_add(qv[:], qv[:], GLN_EPS)
            rsv = _rsqrt_hack(nc, pc, qv[:], 8, "v")  # 0:4 rs_key | 4:8 rs_at
            nc.vector.tensor_copy(b1[:, 4:8], rsv[:, 0:4])
            nc.vector.tensor_copy(b1[:, 12:16], rsv[:, 4:8])
            nc.vector.tensor_tensor(b1[:, 8:12], b1[:, 0:4], rsv[:, 0:4],
                                    OP.mult)
            nc.vector.tensor_tensor(b1[:, 16:20], scr[:, 4:8], rsv[:, 4:8],
                                    OP.mult)

            bn_pair(4)
            bn_pair(5)

            pp_b1 = pps.tile([128, 32], F32, tag="sm")
            nc.tensor.matmul(pp_b1[:, 0:20], oner[:], b1[:], start=True,
                             stop=True)
            bc1 = pc.tile([128, 20], F32, tag="bc1")
            nc.scalar.copy(bc1[:], pp_b1[:, 0:20])

            # ---------------- full-width normalize columns ----------------
            kw4 = pc.tile([128, 4], F32, tag="kw4")
            kb4 = pc.tile([128, 4], F32, tag="kb4")
            cw4 = pc.tile([128, 4], F32, tag="cw4")
            cb4 = pc.tile([128, 4], F32, tag="cb4")
            ct1 = pc.tile([128, 4], F32, tag="ct1")
            ct2 = pc.tile([128, 4], F32, tag="ct2")
            nc.vector.tensor_tensor(kw4[:], gkx4, bc1[:, 4:8], OP.mult)
            nc.vector.tensor_tensor(ct1[:], gkx4, bc1[:, 8:12], OP.mult)
            nc.vector.tensor_tensor(kb4[:], bekx4, ct1[:], OP.subtract)
            nc.vector.tensor_tensor(cw4[:], wax4, bc1[:, 12:16], OP.mult)
            nc.vector.tensor_tensor(ct1[:], ba4x4, bc1[:, 12:16], OP.mult)
            nc.vector.tensor_tensor(ct2[:], ga4x4, bc1[:, 16:20], OP.mult)
            nc.vector.tensor_tensor(ct1[:], ct1[:], ct2[:], OP.subtract)
            nc.vector.tensor_tensor(cb4[:], ct1[:], bea4x4, OP.add)

            # vkln/soft in (ns, b, tv) col order: perm(n) = (n%2)*2 + n//2
            perm = [(n % 2) * 2 + n // 2 for n in range(N)]
            vkln = pc.tile([128, N * TV], F32, tag="vkln")
            vm = pc.tile([128, N * TV], F32, tag="vm")
            for n in range(N):
                nc.vector.tensor_scalar(
                    vkln[:, perm[n] * TV:(perm[n] + 1) * TV],
                    vkf[:, n * TV:(n + 1) * TV],
                    kw4[:, n:n + 1], kb4[:, n:n + 1], OP.mult, OP.add)
                nc.vector.tensor_scalar(
                    vm[:, n * TV:(n + 1) * TV],
                    v1f[:, n * TV:(n + 1) * TV],
                    cw4[:, n:n + 1], cb4[:, n:n + 1], OP.mult, OP.add)
            bn_pair(6)
            bn_pair(7)

            # softmax over tv per (c, n) -- full width
            mx = pc.tile([128, N], F32, tag="mx")
            nc.vector.tensor_reduce(
                mx[:], vm[:].rearrange("p (n t) -> p n t", n=N, t=TV),
                axis=AX.X, op=OP.max)
            nmx = pc.tile([128, N], F32, tag="nmx")
            nc.vector.tensor_scalar_mul(nmx[:], mx[:], -1.0)
            ex = pc.tile([128, N * TV], F32, tag="ex")
            ssum = pc.tile([128, N], F32, tag="ssum")
            for n in range(N):
                nc.scalar.activation(
                    ex[:, n * TV:(n + 1) * TV], vm[:, n * TV:(n + 1) * TV],
                    AF.Exp, bias=nmx[:, n:n + 1],
                    accum_out=ssum[:, n:n + 1])
            rcp = pc.tile([128, N], F32, tag="rcp")
            nc.vector.reciprocal(rcp[:], ssum[:])
            soft = pc.tile([128, N * TV], F32, tag="soft")
            for n in range(N):
                nc.vector.tensor_scalar_mul(
                    soft[:, perm[n] * TV:(perm[n] + 1) * TV],
                    ex[:, n * TV:(n + 1) * TV], rcp[:, n:n + 1])

            # ---------------- transpose to (b,k) x (ns,c) -----------------
            tkey = pc.tile([128, NS * CL], F32, tag="tkey")
            tatt = pc.tile([128, NS * CL], F32, tag="tatt")
            for (src, dst) in ((vkln, tkey), (soft, tatt)):
                for ns in range(NS):
                    pt = pps.tile([128, 32], F32, tag="sm")
                    nc.tensor.matmul(
                        pt[:, 0:CL],
                        src[0:CL, ns * B * TV:(ns + 1) * B * TV],
                        id16[:], start=True, stop=True)
                    nc.scalar.copy(dst[:, ns * CL:(ns + 1) * CL],
                                   pt[:, 0:CL])

            # ============== per-group BN finalize + fused loop ============
            alpha = pc.tile([128, NS * CL], F32, tag="alpha")
            beta = pc.tile([128, NS * CL], F32, tag="beta")
            bcab = pc.tile([128, 64], F32, tag="bcab")
            # bcab row layout per group g (cols g*32..): Av 0:8 | Bv 8:16 |
            #                                            Ag 16:24 | Bg 24:32

            def finalize_group(g):
                c0 = g * GC
                bnsg = bns[:, c0 * 12:(c0 + GC) * 12]
                v4 = bnsg.rearrange("p (c h k) -> p c k h", c=GC, h=4, k=3)
                stk = pc.tile([128, 3 * GC], F32, tag=f"stk{g}")
                # per-partition: sum of the 4 means / 4 cv's / 4 mean^2's
                nc.vector.tensor_reduce(stk[:, 0:GC], v4[:, :, 1:2, :],
                                        axis=AX.X, op=OP.add)
                nc.vector.tensor_reduce(stk[:, GC:2 * GC], v4[:, :, 2:3, :],
                                        axis=AX.X, op=OP.add)
                msq = pc.tile([128, 4 * GC], F32, tag=f"msq{g}")
                mv = msq[:].rearrange("p (c o h) -> p c o h", c=GC, o=1, h=4)
                nc.vector.tensor_tensor(mv[:], v4[:, :, 1:2, :],
                                        v4[:, :, 1:2, :], OP.mult)
                nc.vector.tensor_reduce(stk[:, 2 * GC:3 * GC], mv[:],
                                        axis=AX.X, op=OP.add)
                # cross-partition reduce -> [1, 24]
                pr = pps.tile([128, 32], F32, tag="sm")
                nc.tensor.matmul(pr[0:1, 0:3 * GC], onec[:], stk[:],
                                 start=True, stop=True)
                rr = pc.tile([1, 3 * GC], F32, tag=f"rr{g}")
                nc.scalar.copy(rr[:], pr[0:1, 0:3 * GC])
                # rows: mx = msum/512 ; ex2 = cvsum/NBN + msqsum/512
                mxr = pc.tile([1, 4 * GC], F32, tag=f"mxr{g}")
                # mxr: mx 0:8 | ex2 8:16 | var 16:24 | scratch 24:32
                nc.vector.tensor_scalar_mul(mxr[:, 0:GC], rr[:, 0:GC],
                                            1.0 / 512.0)
                nc.vector.tensor_scalar_mul(mxr[:, 24:32], rr[:, 2 * GC:],
                                            1.0 / 512.0)
                nc.vector.tensor_scalar(mxr[:, 8:16], rr[:, GC:2 * GC],
                                        1.0 / NBN, None, OP.mult)
                nc.vector.tensor_tensor(mxr[:, 8:16], mxr[:, 8:16],
                                        mxr[:, 24:32], OP.add)
                nc.vector.tensor_tensor(mxr[:, 24:32], mxr[:, 0:GC],
                                        mxr[:, 0:GC], OP.mult)
                nc.vector.tensor_tensor(mxr[:, 16:24], mxr[:, 8:16],
                                        mxr[:, 24:32], OP.subtract)
                # qb [1,16]: var*wv2+eps | var*wg2+eps
                qb = pc.tile([1, 2 * GC], F32, tag=f"qb{g}")
                nc.vector.tensor_tensor(qb[:, 0:GC], mxr[:, 16:24],
                                        prow[:, 96 + c0:96 + c0 + GC],
                                        OP.mult)
                nc.vector.tensor_tensor(qb[:, GC:], mxr[:, 16:24],
                                        prow[:, 112 + c0:112 + c0 + GC],
                                        OP.mult)
                nc.vector.tensor_scalar_add(qb[:], qb[:], BN_EPS)
                rsb = _rsqrt_hack(nc, pc, qb[:], 2 * GC, f"b{g}")
                # ab row [1,32]: Av | Bv | Ag | Bg
                ab = pc.tile([1, 32], F32, tag=f"ab{g}")
                nc.vector.tensor_tensor(ab[:, 0:8], rsb[:, 0:8],
                                        prow[:, 16 + c0:16 + c0 + GC],
                                        OP.mult)
                nc.vector.tensor_tensor(ab[:, 0:8], ab[:, 0:8],
                                        prow[:, c0:c0 + GC], OP.mult)
                nc.vector.tensor_tensor(ab[:, 8:16], mxr[:, 0:GC],
                                        ab[:, 0:8], OP.mult)
                nc.vector.tensor_tensor(ab[:, 8:16],
                                        prow[:, 32 + c0:32 + c0 + GC],
                                        ab[:, 8:16], OP.subtract)
                nc.vector.tensor_tensor(ab[:, 16:24], rsb[:, 8:16],
                                        prow[:, 64 + c0:64 + c0 + GC],
                                        OP.mult)
                nc.vector.tensor_tensor(ab[:, 16:24], ab[:, 16:24],
                                        prow[:, 48 + c0:48 + c0 + GC],
                                        OP.mult)
                nc.vector.tensor_tensor(ab[:, 24:32], mxr[:, 0:GC],
                                        ab[:, 16:24], OP.mult)
                nc.vector.tensor_tensor(ab[:, 24:32],
                                        prow[:, 80 + c0:80 + c0 + GC],
                                        ab[:, 24:32], OP.subtract)
                pab = pps.tile([128, 32], F32, tag="sm")
                nc.tensor.matmul(pab[:], oner[:], ab[:], start=True, stop=True)
                bg = bcab[:, g * 32:(g + 1) * 32]
                nc.scalar.copy(bg, pab[:])
                # alpha/beta columns for this group's channels
                for ns in range(NS):
                    asl = slice(ns * CL + c0, ns * CL + c0 + GC)
                    nc.vector.tensor_tensor(alpha[:, asl], tatt[:, asl],
                                            bg[:, 0:8], OP.mult)
                    nc.vector.tensor_tensor(beta[:, asl], tatt[:, asl],
                                            bg[:, 8:16], OP.mult)

            def channel(c):
                g = c // GC
                j = c - g * GC
                bg = bcab[:, g * 32:(g + 1) * 32]
                src = a1c(c)
                # gate = relu(Ag*src + Bg)  (ACT, fp16 out)
                gate = pgate.tile([128, CF], FP16, tag="gate")
                nc.scalar.activation(gate[:], src, AF.Relu,
                                     bias=bg[:, 24 + j:25 + j],
                                     scale=bg[:, 16 + j:17 + j])
                # diagonal weight tiles (DVE, fp16 4x)
                dd = []
                for ns in range(NS):
                    da = pdiag.tile([128, 128], FP16, tag="da")
                    nc.vector.tensor_scalar(
                        da[:], idh[:], alpha[:, ns * CL + c:ns * CL + c + 1],
                        None, OP.mult)
                    dk = pdiag.tile([128, 128], FP16, tag="dk")
                    nc.vector.tensor_scalar(
                        dk[:], idh[:], tkey[:, ns * CL + c:ns * CL + c + 1],
                        None, OP.mult)
                    dd.append((da, dk))
                # PE fuse: P_ns = diag(alpha_ns)@src + diag(key_ns)@gate
                if c % 2 == 0:
                    channel.ost = post.tile([128, 2 * NS * CF], F32,
                                            tag="ost")
                ost = channel.ost
                base = (c % 2) * NS * CF
                for ns in range(NS):
                    da, dk = dd[ns]
                    pt = ppo.tile([128, CF], F32, tag="pfuse")
                    for hh in range(2):
                        sl = slice(hh * 512, (hh + 1) * 512)
                        nc.tensor.matmul(pt[:, sl], da[:], src[:, sl],
                                         start=True, stop=False)
                    for hh in range(2):
                        sl = slice(hh * 512, (hh + 1) * 512)
                        nc.tensor.matmul(pt[:, sl], dk[:], gate[:, sl],
                                         start=False, stop=True)
                    # PSUM -> SBUF copy with beta bias
                    dst = ost[:, base + ns * CF:base + (ns + 1) * CF]
                    bcol = beta[:, ns * CL + c:ns * CL + c + 1]
                    if (c, ns) in DVE_COPIES:
                        nc.vector.tensor_scalar(dst, pt[:], 1.0, bcol,
                                                OP.mult, OP.add)
                    else:
                        nc.scalar.activation(dst, pt[:], AF.Identity,
                                             bias=bcol, scale=1.0)
                if c % 2 == 1:
                    nc.sync.dma_start(
                        d_out.ap()[:, (c - 1) * NS * CF:(c + 1) * NS * CF],
                        ost[:])

            with tc.tile_wait_until(0.011):
                finalize_group(0)
            for c in range(0, 8):
                with tc.tile_wait_until(0.013 + 0.0022 * c):
                    channel(c)
                bn_pair(8 + c)
            with tc.tile_wait_until(0.032):
                finalize_group(1)
            for c in range(8, 16):
                with tc.tile_wait_until(0.034 + 0.0022 * (c - 8)):
                    channel(c)

    nc.compile()
    return nc


_NC_CACHE = None


def _get_nc():
    global _NC_CACHE
    if _NC_CACHE is None:
        _NC_CACHE = _build()
    return _NC_CACHE


def _pack_inputs(a1, v1, w_gate, b_gate, g_gate, be_gate,
                 w_val, b_val, g_val, be_val,
                 w_attn, b_attn, g_attn, be_attn,
                 w_key, b_key, g_key, be_key):
    f32 = np.float32
    a1 = np.asarray(a1, f32)
    v1 = np.asarray(v1, f32)
    v1f = np.ascontiguousarray(v1.transpose(1, 0, 2).reshape(CA, N * TV))
    wa2 = np.asarray(w_attn, f32).reshape(CA, H)
    ba2 = np.asarray(b_attn, f32).reshape(CA, H)
    ga2 = np.asarray(g_attn, f32).reshape(CA, H)
    bea2 = np.asarray(be_attn, f32).reshape(CA, H)
    wk = np.asarray(w_key, f32)
    bk = np.asarray(b_key, f32)
    gk = np.asarray(g_key, f32)
    bek = np.asarray(be_key, f32)
    swa = wa2.sum(1)
    sw2 = (wa2 * wa2).sum(1)
    swb2 = 2.0 * (wa2 * ba2).sum(1)
    wacol = (wa2 * ga2).sum(1) * 0.25
    ba4 = (ba2 * ga2).sum(1) * 0.25
    ga4s = ga2.sum(1) * 0.25
    bea4 = bea2.sum(1) * 0.25
    sba = np.full(4, TV * ba2.sum(), f32)
    sba2 = np.full(4, TV * (ba2 * ba2).sum(), f32)
    id16 = np.eye(CL, dtype=f32)
    idh = np.eye(128, dtype=np.float16)

    def x4(v):
        return np.repeat(v[:, None], 4, axis=1)

    in_maps = []
    for i in range(NCORE):
        sl = slice(i * CL, (i + 1) * CL)
        x = a1[:, sl].reshape(B, CL, TV, RP, FQ)
        x = np.ascontiguousarray(x.transpose(0, 2, 1, 3, 4))
        a1s = x.reshape(128, AFREE).astype(np.float16)
        # channel permutation: this core's channels first
        pidx = np.concatenate([np.arange(i * CL, (i + 1) * CL),
                               np.arange(0, i * CL),
                               np.arange((i + 1) * CL, CA)])
        cb1 = np.concatenate(
            [v1f[pidx],
             wk[pidx, None], bk[pidx, None], swa[pidx, None],
             sw2[pidx, None], swb2[pidx, None],
             x4(gk[pidx]), x4(bek[pidx]), x4(wacol[pidx]),
             x4(ba4[pidx]), x4(ga4s[pidx]), x4(bea4[pidx])], axis=1)
        cb1 = np.ascontiguousarray(cb1)
        wv = np.asarray(w_val, f32)[sl]
        wg = np.asarray(w_gate, f32)[sl]
        prow = np.concatenate(
            [wv, np.asarray(g_val, f32)[sl],
             np.asarray(be_val, f32)[sl], wg,
             np.asarray(g_gate, f32)[sl],
             np.asarray(be_gate, f32)[sl],
             wv * wv, wg * wg])[None, :]
        cb3 = np.ascontiguousarray(
            np.concatenate([np.ones((1, 128), f32), prow,
                            sba[None, :], sba2[None, :]], axis=1))
        in_maps.append({"a1s": a1s, "cb1": cb1, "cb2": id16, "cb3": cb3,
                        "cbh": idh})
    return in_maps


def _unpack_output(results):
    out = np.empty((N, CA, T, FQ), np.float32)
    for i in range(NCORE):
        r = np.asarray(results[i]["out"]).reshape(B, TV, CL, NS, RP, FQ)
        r = r.transpose(0, 3, 2, 1, 4, 5).reshape(N, CL, T, FQ)
        out[:, i * CL:(i + 1) * CL] = r
    return out


def _install_ntff_shim():
    """The agent image's ``antenv`` lacks ``axon_hooks``; recreate it and
    register the ctypes NTFF hook against /opt/axon/libaxon_pjrt.so (the
    same mechanism trn_boot uses when the module exists)."""
    import sys
    import types
    import ctypes
    import contextlib

    if "antenv.axon_hooks" in sys.modules:
        return True
    so_path = "/opt/axon/libaxon_pjrt.so"
    try:
        lib = ctypes.CDLL(so_path)
    except OSError:
        return False
    if not hasattr(lib, "axon_start_nrt_profile"):
        return False
    lib.axon_start_nrt_profile.argtypes = [ctypes.POINTER(ctypes.c_int64),
                                           ctypes.c_size_t]
    lib.axon_start_nrt_profile.restype = ctypes.c_int64
    lib.axon_stop_nrt_profile.argtypes = [ctypes.c_char_p]
    lib.axon_stop_nrt_profile.restype = ctypes.c_int64

    @contextlib.contextmanager
    def _hook(output_dir, device_ids):
        import jax
        jax.devices()
        if device_ids:
            ids = (ctypes.c_int64 * len(device_ids))(*device_ids)
            rc = lib.axon_start_nrt_profile(ids, len(device_ids))
        else:
            rc = lib.axon_start_nrt_profile(None, 0)
        if rc != 0:
            raise RuntimeError(f"axon_start_nrt_profile rc={rc}")
        try:
            yield
        finally:
            n = lib.axon_stop_nrt_profile(str(output_dir).encode())
            print(f"profile: {n} file(s) written to {output_dir}",
                  file=sys.stderr)

    mod = types.ModuleType("antenv.axon_hooks")
    _state = {"hook": _hook}
    mod.get_axon_ntff_profile_hook = lambda: _state["hook"]

    def set_axon_ntff_profile_hook(h):
        _state["hook"] = h

    mod.set_axon_ntff_profile_hook = set_axon_ntff_profile_hook
    import antenv
    antenv.axon_hooks = mod
    sys.modules["antenv.axon_hooks"] = mod
    return True


def run(inputs, trace=False, **trace_kwargs):
    """Returns (output, BassKernelResults)."""
    nc = _get_nc()
    in_maps = _pack_inputs(**inputs)
    if trace and not _install_ntff_shim():
        trace = False
    br = run_bass_kernel_spmd(nc, in_maps, core_ids=list(range(NCORE)),
                              trace=trace, **trace_kwargs)
    return _unpack_output(br.results), br


def kernel(**inputs):
    out, _ = run(inputs)
    return out


# revision 16
# speedup vs baseline: 1.0731x; 1.0731x over previous
"""Bass/Tile Trainium2 kernel for the CAFBlock fusion (nn_CAFBlock).

Strategy: shard the audio channel dim C_a=128 across 8 NeuronCores (16
channels per core).  BatchNorm2d statistics are per-channel -> fully local.
The tiny video branch (gLN over all channels) is computed redundantly on
every core from a replicated copy of v1, so there are no collectives.

Per-core SBUF layout for the big tensors: partition p = b*64 + k where k is
the video-frame index (t = k*8 + r), free dim = (c_local, r, f).  With this
layout the nearest-interpolated v_attn/v_key factors are constant along the
free dim, so the fused output

    out[ns,c] = (attn_ns*Av)ated * src + key_ns * relu(Ag*src+Bg) + attn_ns*Bv

is computed on the *tensor engine* as two accumulated diagonal matmuls per
(ns, half): diag(alpha_ns) @ src + diag(key_ns) @ gate, with the rank-1
beta term folded into the PSUM->SBUF copy as a per-partition bias.  a1 is
shipped to the device in fp16 (halves input DMA traffic; rel err ~5e-4 vs
the 2e-2 gate).  BN statistics come from per-channel bn_stats pairs; all
rsqrts use an integer-Newton iteration on the DVE so the scalar engine
loads exactly one activation table (exp, for the softmax).
"""

import numpy as np

import concourse.bass as bass
import concourse.bacc as bacc
import concourse.tile as tile
import concourse.mybir as mybir
from concourse.bass_utils import run_bass_kernel_spmd

F32 = mybir.dt.float32
FP16 = mybir.dt.float16
I32 = mybir.dt.int32
AF = mybir.ActivationFunctionType
OP = mybir.AluOpType
AX = mybir.AxisListType
MS = bass.MemorySpace

# problem dims (hardcoded per the harness contract)
B, NS, CA, H, T, FQ, TV = 2, 2, 128, 4, 512, 128, 64
NCORE = 8
CL = CA // NCORE            # 16 local channels per core
N = B * NS                  # 4 (b*ns video samples)
RP = T // TV                # 8 (nearest-interp repeat factor)
BN_EPS, GLN_EPS = 1e-5, 1e-8
NBN = float(B * T * FQ)     # 131072 elements per BN channel
NKEY = float(CA * TV)       # 8192 elements per gLN(key) sample
NATT = float(CA * H * TV)   # 32768 elements per gLN(attn) sample
CF = RP * FQ                # 1024 free elements per channel tile
AFREE = CL * CF             # 16384 free elements of resident a1 shard
OFREE = CL * NS * CF        # 32768 free elements of output
GC = 4                      # channels per finalize group
K_MAGIC = 0x5F3759DF

# (c, ns) pairs whose PSUM->SBUF copy runs on the DVE instead of ACT
DVE_COPIES = frozenset(
    (c, 1) for c in (0, 2, 3, 5, 6, 8, 9, 11, 12, 14))


def _rsqrt_hack(nc, pc, q, width, pref):
    """y = 1/sqrt(q) for positive q via int bit-hack + 3 Newton steps.

    Runs entirely on the DVE (no activation tables)."""
    kcol = pc.tile([1, width], I32, tag=pref + "kc")
    t1 = pc.tile([1, width], I32, tag=pref + "t1")
    y = pc.tile([1, width], F32, tag=pref + "y")
    ysq = pc.tile([1, width], F32, tag=pref + "ys")
    nc.vector.memset(kcol[:], K_MAGIC)
    nc.vector.tensor_scalar(t1[:], q.bitcast(I32), 1, None,
                            OP.logical_shift_right)
    nc.vector.tensor_tensor(y[:].bitcast(I32), kcol[:], t1[:], OP.subtract)
    for _ in range(2):
        nc.vector.tensor_tensor(ysq[:], y[:], y[:], OP.mult)
        nc.vector.tensor_tensor(ysq[:], q, ysq[:], OP.mult)
        nc.vector.tensor_scalar(ysq[:], ysq[:], -0.5, 1.5, OP.mult, OP.add)
        nc.vector.tensor_tensor(y[:], y[:], ysq[:], OP.mult)
    return y


def _build():
    """Builds the SPMD Bass program (same program on all 8 cores)."""
    nc = bacc.Bacc("TRN2", target_bir_lowering=False, debug=False)

    d_a1 = nc.dram_tensor("a1s", [128, AFREE], FP16, kind="ExternalInput")
    # consts packed host-side:
    # cb1 [128, 266]: v1f 0:256 | pcol 256:266
    # cb2 [16, 290]:  v1l 0:256 | ploc 256:274 | id16 274:290
    # cb3 [1, 257]:   oner 0:128 | prow 128:256 | onec-col via memset
    # cbh [128, 128] fp16: identity
    d_cb1 = nc.dram_tensor("cb1", [128, 285], F32, kind="ExternalInput")
    d_cb2 = nc.dram_tensor("cb2", [CL, CL], F32, kind="ExternalInput")
    d_cb3 = nc.dram_tensor("cb3", [1, 264], F32, kind="ExternalInput")
    d_cbh = nc.dram_tensor("cbh", [128, 128], FP16, kind="ExternalInput")
    d_out = nc.dram_tensor("out", [128, OFREE], F32, kind="ExternalOutput")

    with tile.TileContext(nc) as tc:
        with (
            tc.tile_pool(name="pout", bufs=3, space=MS.PSUM) as ppo,
            tc.tile_pool(name="psmall", bufs=2, space=MS.PSUM) as pps,
            tc.tile_pool(name="pres", bufs=8) as pres,
            tc.tile_pool(name="pconst", bufs=1) as pc,
            tc.tile_pool(name="pgate", bufs=3) as pgate,
            tc.tile_pool(name="pdiag", bufs=8) as pdiag,
            tc.tile_pool(name="post", bufs=3) as post,
        ):
            # ---------------- constants (4 fast HWDGE DMAs) ---------------
            cb1 = pc.tile([128, 285], F32, tag="cb1")
            cb2 = pc.tile([CL, CL], F32, tag="cb2")
            cb3 = pc.tile([1, 264], F32, tag="cb3")
            idh = pc.tile([128, 128], FP16, tag="idh")
            res = []
            for g in range(8):
                rt = pres.tile([128, 2048], FP16, tag="res")
                res.append(rt)
            # parallel DMA issue: SP carries cb1 + first tiles, ACT the
            # remaining consts, Pool (SWDGE) the tail tiles
            nc.sync.dma_start(cb1[:], d_cb1.ap()[:])
            for g in range(0, 5):
                nc.sync.dma_start(res[g][:],
                                  d_a1.ap()[:, g * 2048:(g + 1) * 2048])
            nc.scalar.dma_start(cb3[:], d_cb3.ap()[:])
            nc.scalar.dma_start(cb2[:], d_cb2.ap()[:])
            nc.scalar.dma_start(idh[:], d_cbh.ap()[:])
            for g in range(5, 8):
                nc.gpsimd.dma_start(res[g][:],
                                    d_a1.ap()[:, g * 2048:(g + 1) * 2048])
            v1f = cb1[:, 0:256]
            wkc = cb1[:, 256:257]
            bkc = cb1[:, 257:258]
            swac = cb1[:, 258:259]
            sw2c = cb1[:, 259:260]
            swb2c = cb1[:, 260:261]
            gkx4 = cb1[:, 261:265]
            bekx4 = cb1[:, 265:269]
            wax4 = cb1[:, 269:273]
            ba4x4 = cb1[:, 273:277]
            ga4x4 = cb1[:, 277:281]
            bea4x4 = cb1[:, 281:285]
            id16 = cb2[:, 0:CL]
            oner = cb3[:, 0:128]
            prow = cb3[:, 128:256]
            sbax4 = cb3[:, 256:260]
            sba2x4 = cb3[:, 260:264]
            # prow: wv 0:16 | gv 16:32 | bev 32:48 | wg 48:64 | gg 64:80 |
            #       beg 80:96 | wv2 96:112 | wg2 112:128
            onec = pc.tile([128, 1], F32, tag="onec")
            nc.vector.memset(onec[:], 1.0)

            def a1c(c):
                return res[c // 2][:, (c % 2) * CF:(c % 2) * CF + CF]

            # ============== v-branch elementwise (DVE, latency-critical) ==
            # vkf = v1*wk + bk (per perm'd channel); squares + affine-attn
            vkf = pc.tile([128, N * TV], F32, tag="vkf")
            scrk = pc.tile([128, N * TV], F32, tag="scrk")
            uswa = pc.tile([128, N * TV], F32, tag="uswa")
            v1sq = pc.tile([128, N * TV], F32, tag="v1sq")
            za = pc.tile([128, N * TV], F32, tag="za")
            zc = pc.tile([128, N * TV], F32, tag="zc")
            zd = pc.tile([128, N * TV], F32, tag="zd")
            if True:
                nc.vector.tensor_scalar(vkf[:], v1f[:], wkc, bkc, OP.mult,
                                        OP.add)
                nc.vector.tensor_tensor(scrk[:], vkf[:], vkf[:], OP.mult)
                nc.vector.tensor_scalar(uswa[:], v1f[:], swac, None, OP.mult)
                nc.vector.tensor_tensor(v1sq[:], v1f[:], v1f[:], OP.mult)
                nc.vector.tensor_scalar(za[:], v1sq[:], sw2c, None, OP.mult)
                nc.vector.tensor_scalar(zc[:], v1f[:], swb2c, None, OP.mult)
                nc.vector.tensor_tensor(zd[:], za[:], zc[:], OP.add)

            # ============== BN stats c=0..3 (DVE) =========================
            bns = pc.tile([128, CL * 12], F32, tag="bns")

            def bn_pair(c):
                src = a1c(c)
                floor = (0.0115 + 0.0013 * c if c < 8
                         else 0.022 + 0.0018 * (c - 8))
                with tc.tile_wait_until(floor):
                    nc.vector.bn_stats(bns[:, c * 12:c * 12 + 6],
                                       src[:, 0:512])
                    nc.vector.bn_stats(bns[:, c * 12 + 6:c * 12 + 12],
                                       src[:, 512:1024])


            # ============== v-branch reductions (DVE) =====================
            red = pc.tile([128, 16], F32, tag="red")
            for (dst0, srcv) in ((0, vkf), (4, scrk), (8, uswa), (12, zd)):
                nc.vector.tensor_reduce(
                    red[:, dst0:dst0 + 4],
                    srcv[:].rearrange("p (n t) -> p n t", n=N, t=TV),
                    axis=AX.X, op=OP.add)


            bn_pair(0)
            bn_pair(1)

            # ------- v-branch cross-partition reduction + finalize --------
            pp_ks = pps.tile([128, 32], F32, tag="sm")
            nc.tensor.matmul(pp_ks[0:1, 0:16], onec[:], red[:], start=True,
                             stop=True)
            rr = pc.tile([1, 16], F32, tag="rr")
            nc.scalar.copy(rr[:], pp_ks[0:1, 0:16])
            # rr: sum_vkf 0:4 | sum_vkf2 4:8 | sum_u 8:12 | sum_z 12:16

            bn_pair(2)
            bn_pair(3)

            # b1 [1,20]: km 0:4 | rs_k 4:8 | km*rs_k 8:12 | rs_a 12:16 |
            #            am*rs_a 16:20
            b1 = pc.tile([1, 20], F32, tag="b1")
            scr = pc.tile([1, 12], F32, tag="scr")   # kE2 0:4 | am 4:8 | aE2 8:12
            qv = pc.tile([1, 8], F32, tag="qv")
            nc.vector.tensor_scalar_mul(b1[:, 0:4], rr[:, 0:4], 1.0 / NKEY)
            nc.vector.tensor_scalar_mul(scr[:, 0:4], rr[:, 4:8], 1.0 / NKEY)
            nc.vector.tensor_tensor(qv[:, 0:4], b1[:, 0:4], b1[:, 0:4],
                                    OP.mult)
            nc.vector.tensor_tensor(qv[:, 0:4], scr[:, 0:4], qv[:, 0:4],
                                    OP.subtract)
            nc.vector.tensor_tensor(scr[:, 4:8], rr[:, 8:12], sbax4, OP.add)
            nc.vector.tensor_scalar_mul(scr[:, 4:8], scr[:, 4:8], 1.0 / NATT)
            nc.vector.tensor_tensor(scr[:, 8:12], rr[:, 12:16], sba2x4,
                                    OP.add)
            nc.vector.tensor_scalar_mul(scr[:, 8:12], scr[:, 8:12],
                                        1.0 / NATT)
            nc.vector.tensor_tensor(qv[:, 4:8], scr[:, 4:8], scr[:, 4:8],
                                    OP.mult)
            nc.vector.tensor_tensor(qv[:, 4:8], scr[:, 8:12], qv[:, 4:8],
                                    OP.subtract)
            nc.vector.tensor_scalar_add(qv[:], qv[:], GLN_EPS)
            rsv = _rsqrt_hack(nc, pc, qv[:], 8, "v")  # 0:4 rs_key | 4:8 rs_at
            nc.vector.tensor_copy(b1[:, 4:8], rsv[:, 0:4])
            nc.vector.tensor_copy(b1[:, 12:16], rsv[:, 4:8])
            nc.vector.tensor_tensor(b1[:, 8:12], b1[:, 0:4], rsv[:, 0:4],
                                    OP.mult)
            nc.vector.tensor_tensor(b1[:, 16:20], scr[:, 4:8], rsv[:, 4:8],
                                    OP.mult)

            bn_pair(4)
            bn_pair(5)

            pp_b1 = pps.tile([128, 32], F32, tag="sm")
            nc.tensor.matmul(pp_b1[:, 0:20], oner[:], b1[:], start=True,
                             stop=True)
            bc1 = pc.tile([128, 20], F32, tag="bc1")
            nc.scalar.copy(bc1[:], pp_b1[:, 0:20])

            # ---------------- full-width normalize columns ----------------
            kw4 = pc.tile([128, 4], F32, tag="kw4")
            kb4 = pc.tile([128, 4], F32, tag="kb4")
            cw4 = pc.tile([128, 4], F32, tag="cw4")
            cb4 = pc.tile([128, 4], F32, tag="cb4")
            ct1 = pc.tile([128, 4], F32, tag="ct1")
            ct2 = pc.tile([128, 4], F32, tag="ct2")
            nc.vector.tensor_tensor(kw4[:], gkx4, bc1[:, 4:8], OP.mult)
            nc.vector.tensor_tensor(ct1[:], gkx4, bc1[:, 8:12], OP.mult)
            nc.vector.tensor_tensor(kb4[:], bekx4, ct1[:], OP.subtract)
            nc.vector.tensor_tensor(cw4[:], wax4, bc1[:, 12:16], OP.mult)
            nc.vector.tensor_tensor(ct1[:], ba4x4, bc1[:, 12:16], OP.mult)
            nc.vector.tensor_tensor(ct2[:], ga4x4, bc1[:, 16:20], OP.mult)
            nc.vector.tensor_tensor(ct1[:], ct1[:], ct2[:], OP.subtract)
            nc.vector.tensor_tensor(cb4[:], ct1[:], bea4x4, OP.add)

            # vkln/soft in (ns, b, tv) col order: perm(n) = (n%2)*2 + n//2
            perm = [(n % 2) * 2 + n // 2 for n in range(N)]
            vkln = pc.tile([128, N * TV], F32, tag="vkln")
            vm = pc.tile([128, N * TV], F32, tag="vm")
            for n in range(N):
                nc.vector.tensor_scalar(
                    vkln[:, perm[n] * TV:(perm[n] + 1) * TV],
                    vkf[:, n * TV:(n + 1) * TV],
                    kw4[:, n:n + 1], kb4[:, n:n + 1], OP.mult, OP.add)
                nc.vector.tensor_scalar(
                    vm[:, n * TV:(n + 1) * TV],
                    v1f[:, n * TV:(n + 1) * TV],
                    cw4[:, n:n + 1], cb4[:, n:n + 1], OP.mult, OP.add)
            bn_pair(6)
            bn_pair(7)

            # softmax over tv per (c, n) -- full width
            mx = pc.tile([128, N], F32, tag="mx")
            nc.vector.tensor_reduce(
                mx[:], vm[:].rearrange("p (n t) -> p n t", n=N, t=TV),
                axis=AX.X, op=OP.max)
            nmx = pc.tile([128, N], F32, tag="nmx")
            nc.vector.tensor_scalar_mul(nmx[:], mx[:], -1.0)
            ex = pc.tile([128, N * TV], F32, tag="ex")
            ssum = pc.tile([128, N], F32, tag="ssum")
            for n in range(N):
                nc.scalar.activation(
                    ex[:, n * TV:(n + 1) * TV], vm[:, n * TV:(n + 1) * TV],
                    AF.Exp, bias=nmx[:, n:n + 1],
                    accum_out=ssum[:, n:n + 1])
            rcp = pc.tile([128, N], F32, tag="rcp")
            nc.vector.reciprocal(rcp[:], ssum[:])
            soft = pc.tile([128, N * TV], F32, tag="soft")
            for n in range(N):
                nc.vector.tensor_scalar_mul(
                    soft[:, perm[n] * TV:(perm[n] + 1) * TV],
                    ex[:, n * TV:(n + 1) * TV], rcp[:, n:n + 1])

            # ---------------- transpose to (b,k) x (ns,c) -----------------
            tkey = pc.tile([128, NS * CL], F32, tag="tkey")
            tatt = pc.tile([128, NS * CL], F32, tag="tatt")
            for (src, dst) in ((vkln, tkey), (soft, tatt)):
                for ns in range(NS):
                    pt = pps.tile([128, 32], F32, tag="sm")
                    nc.tensor.matmul(
                        pt[:, 0:CL],
                        src[0:CL, ns * B * TV:(ns + 1) * B * TV],
                        id16[:], start=True, stop=True)
                    nc.scalar.copy(dst[:, ns * CL:(ns + 1) * CL],
                                   pt[:, 0:CL])

            # ============== per-group BN finalize + fused loop ============
            alpha = pc.tile([128, NS * CL], F32, tag="alpha")
            beta = pc.tile([128, NS * CL], F32, tag="beta")
            bcab = pc.tile([128, 4 * CL], F32, tag="bcab")
            # bcab layout per group g (cols g*4*GC..):
            #   Av 0:GC | Bv GC:2GC | Ag 2GC:3GC | Bg 3GC:4GC

            def finalize_group(g):
                c0 = g * GC
                W = GC
                bnsg = bns[:, c0 * 12:(c0 + GC) * 12]
                v4 = bnsg.rearrange("p (c h k) -> p c k h", c=GC, h=4, k=3)
                stk = pc.tile([128, 3 * GC], F32, tag=f"stk{g}")
                # per-partition: sum of the 4 means / 4 cv's / 4 mean^2's
                nc.vector.tensor_reduce(stk[:, 0:W], v4[:, :, 1:2, :],
                                        axis=AX.X, op=OP.add)
                nc.vector.tensor_reduce(stk[:, W:2 * W], v4[:, :, 2:3, :],
                                        axis=AX.X, op=OP.add)
                msq = pc.tile([128, 4 * GC], F32, tag=f"msq{g}")
                mv = msq[:].rearrange("p (c o h) -> p c o h", c=GC, o=1, h=4)
                nc.vector.tensor_tensor(mv[:], v4[:, :, 1:2, :],
                                        v4[:, :, 1:2, :], OP.mult)
                nc.vector.tensor_reduce(stk[:, 2 * W:3 * W], mv[:],
                                        axis=AX.X, op=OP.add)
                # cross-partition reduce -> [1, 3W]
                pr = pps.tile([128, 32], F32, tag="sm")
                nc.tensor.matmul(pr[0:1, 0:3 * W], onec[:], stk[:],
                                 start=True, stop=True)
                rr = pc.tile([1, 3 * W], F32, tag=f"rr{g}")
                nc.scalar.copy(rr[:], pr[0:1, 0:3 * W])
                # rows: mx = msum/512 ; ex2 = cvsum/NBN + msqsum/512
                mxr = pc.tile([1, 4 * W], F32, tag=f"mxr{g}")
                # mxr: mx | ex2 | var | scratch  (W each)
                nc.vector.tensor_scalar_mul(mxr[:, 0:W], rr[:, 0:W],
                                            1.0 / 512.0)
                nc.vector.tensor_scalar_mul(mxr[:, 3 * W:], rr[:, 2 * W:],
                                            1.0 / 512.0)
                nc.vector.tensor_scalar(mxr[:, W:2 * W], rr[:, W:2 * W],
                                        1.0 / NBN, None, OP.mult)
                nc.vector.tensor_tensor(mxr[:, W:2 * W], mxr[:, W:2 * W],
                                        mxr[:, 3 * W:], OP.add)
                nc.vector.tensor_tensor(mxr[:, 3 * W:], mxr[:, 0:W],
                                        mxr[:, 0:W], OP.mult)
                nc.vector.tensor_tensor(mxr[:, 2 * W:3 * W], mxr[:, W:2 * W],
                                        mxr[:, 3 * W:], OP.subtract)
                # qb: var*wv2+eps | var*wg2+eps
                qb = pc.tile([1, 2 * W], F32, tag=f"qb{g}")
                nc.vector.tensor_tensor(qb[:, 0:W], mxr[:, 2 * W:3 * W],
                                        prow[:, 96 + c0:96 + c0 + W],
                                        OP.mult)
                nc.vector.tensor_tensor(qb[:, W:], mxr[:, 2 * W:3 * W],
                                        prow[:, 112 + c0:112 + c0 + W],
                                        OP.mult)
                nc.vector.tensor_scalar_add(qb[:], qb[:], BN_EPS)
                rsb = _rsqrt_hack(nc, pc, qb[:], 2 * W, f"b{g}")
                # ab row: Av | Bv | Ag | Bg (W each)
                ab = pc.tile([1, 4 * W], F32, tag=f"ab{g}")
                nc.vector.tensor_tensor(ab[:, 0:W], rsb[:, 0:W],
                                        prow[:, 16 + c0:16 + c0 + W],
                                        OP.mult)
                nc.vector.tensor_tensor(ab[:, 0:W], ab[:, 0:W],
                                        prow[:, c0:c0 + W], OP.mult)
                nc.vector.tensor_tensor(ab[:, W:2 * W], mxr[:, 0:W],
                                        ab[:, 0:W], OP.mult)
                nc.vector.tensor_tensor(ab[:, W:2 * W],
                                        prow[:, 32 + c0:32 + c0 + W],
                                        ab[:, W:2 * W], OP.subtract)
                nc.vector.tensor_tensor(ab[:, 2 * W:3 * W], rsb[:, W:2 * W],
                                        prow[:, 64 + c0:64 + c0 + W],
                                        OP.mult)
                nc.vector.tensor_tensor(ab[:, 2 * W:3 * W],
                                        ab[:, 2 * W:3 * W],
                                        prow[:, 48 + c0:48 + c0 + W],
                                        OP.mult)
                nc.vector.tensor_tensor(ab[:, 3 * W:], mxr[:, 0:W],
                                        ab[:, 2 * W:3 * W], OP.mult)
                nc.vector.tensor_tensor(ab[:, 3 * W:],
                                        prow[:, 80 + c0:80 + c0 + W],
                                        ab[:, 3 * W:], OP.subtract)
                pab = pps.tile([128, 32], F32, tag="sm")
                nc.tensor.matmul(pab[:, 0:4 * W], oner[:], ab[:], start=True,
                                 stop=True)
                bg = bcab[:, g * 4 * GC:(g + 1) * 4 * GC]
                nc.scalar.copy(bg, pab[:, 0:4 * W])
                # alpha/beta columns for this group's channels
                for ns in range(NS):
                    asl = slice(ns * CL + c0, ns * CL + c0 + GC)
                    nc.vector.tensor_tensor(alpha[:, asl], tatt[:, asl],
                                            bg[:, 0:W], OP.mult)
                    nc.vector.tensor_tensor(beta[:, asl], tatt[:, asl],
                                            bg[:, W:2 * W], OP.mult)

            def channel(c):
                g = c // GC
                j = c - g * GC
                bg = bcab[:, g * 4 * GC:(g + 1) * 4 * GC]
                src = a1c(c)
                # gate = relu(Ag*src + Bg)  (ACT, fp16 out)
                gate = pgate.tile([128, CF], FP16, tag="gate")
                nc.scalar.activation(gate[:], src, AF.Relu,
                                     bias=bg[:, 3 * GC + j:3 * GC + j + 1],
                                     scale=bg[:, 2 * GC + j:2 * GC + j + 1])
                # diagonal weight tiles (DVE, fp16 4x)
                dd = []
                for ns in range(NS):
                    da = pdiag.tile([128, 128], FP16, tag="da")
                    nc.vector.tensor_scalar(
                        da[:], idh[:], alpha[:, ns * CL + c:ns * CL + c + 1],
                        None, OP.mult)
                    dk = pdiag.tile([128, 128], FP16, tag="dk")
                    nc.vector.tensor_scalar(
                        dk[:], idh[:], tkey[:, ns * CL + c:ns * CL + c + 1],
                        None, OP.mult)
                    dd.append((da, dk))
                # PE fuse: P_ns = diag(alpha_ns)@src + diag(key_ns)@gate
                if c % 2 == 0:
                    channel.ost = post.tile([128, 2 * NS * CF], F32,
                                            tag="ost")
                ost = channel.ost
                base = (c % 2) * NS * CF
                for ns in range(NS):
                    da, dk = dd[ns]
                    pt = ppo.tile([128, CF], F32, tag="pfuse")
                    for hh in range(2):
                        sl = slice(hh * 512, (hh + 1) * 512)
                        nc.tensor.matmul(pt[:, sl], da[:], src[:, sl],
                                         start=True, stop=False)
                    for hh in range(2):
                        sl = slice(hh * 512, (hh + 1) * 512)
                        nc.tensor.matmul(pt[:, sl], dk[:], gate[:, sl],
                                         start=False, stop=True)
                    # PSUM -> SBUF copy with beta bias
                    dst = ost[:, base + ns * CF:base + (ns + 1) * CF]
                    bcol = beta[:, ns * CL + c:ns * CL + c + 1]
                    if (c, ns) in DVE_COPIES:
                        nc.vector.tensor_scalar(dst, pt[:], 1.0, bcol,
                                                OP.mult, OP.add)
                    else:
                        nc.scalar.activation(dst, pt[:], AF.Identity,
                                             bias=bcol, scale=1.0)
                if c % 2 == 1:
                    nc.sync.dma_start(
                        d_out.ap()[:, (c - 1) * NS * CF:(c + 1) * NS * CF],
                        ost[:])

            for g in range(4):
                with tc.tile_wait_until(0.012 + 0.006 * g):
                    finalize_group(g)
                for c in range(g * 4, (g + 1) * 4):
                    with tc.tile_wait_until(0.013 + 0.0023 * c):
                        channel(c)
                    if c + 8 < 16:
                        bn_pair(c + 8)

    nc.compile()
    return nc


_NC_CACHE = None


def _get_nc():
    global _NC_CACHE
    if _NC_CACHE is None:
        _NC_CACHE = _build()
    return _NC_CACHE


def _pack_inputs(a1, v1, w_gate, b_gate, g_gate, be_gate,
                 w_val, b_val, g_val, be_val,
                 w_attn, b_attn, g_attn, be_attn,
                 w_key, b_key, g_key, be_key):
    f32 = np.float32
    a1 = np.asarray(a1, f32)
    v1 = np.asarray(v1, f32)
    v1f = np.ascontiguousarray(v1.transpose(1, 0, 2).reshape(CA, N * TV))
    wa2 = np.asarray(w_attn, f32).reshape(CA, H)
    ba2 = np.asarray(b_attn, f32).reshape(CA, H)
    ga2 = np.asarray(g_attn, f32).reshape(CA, H)
    bea2 = np.asarray(be_attn, f32).reshape(CA, H)
    wk = np.asarray(w_key, f32)
    bk = np.asarray(b_key, f32)
    gk = np.asarray(g_key, f32)
    bek = np.asarray(be_key, f32)
    swa = wa2.sum(1)
    sw2 = (wa2 * wa2).sum(1)
    swb2 = 2.0 * (wa2 * ba2).sum(1)
    wacol = (wa2 * ga2).sum(1) * 0.25
    ba4 = (ba2 * ga2).sum(1) * 0.25
    ga4s = ga2.sum(1) * 0.25
    bea4 = bea2.sum(1) * 0.25
    sba = np.full(4, TV * ba2.sum(), f32)
    sba2 = np.full(4, TV * (ba2 * ba2).sum(), f32)
    id16 = np.eye(CL, dtype=f32)
    idh = np.eye(128, dtype=np.float16)

    def x4(v):
        return np.repeat(v[:, None], 4, axis=1)

    in_maps = []
    for i in range(NCORE):
        sl = slice(i * CL, (i + 1) * CL)
        x = a1[:, sl].reshape(B, CL, TV, RP, FQ)
        x = np.ascontiguousarray(x.transpose(0, 2, 1, 3, 4))
        a1s = x.reshape(128, AFREE).astype(np.float16)
        # channel permutation: this core's channels first
        pidx = np.concatenate([np.arange(i * CL, (i + 1) * CL),
                               np.arange(0, i * CL),
                               np.arange((i + 1) * CL, CA)])
        cb1 = np.concatenate(
            [v1f[pidx],
             wk[pidx, None], bk[pidx, None], swa[pidx, None],
             sw2[pidx, None], swb2[pidx, None],
             x4(gk[pidx]), x4(bek[pidx]), x4(wacol[pidx]),
             x4(ba4[pidx]), x4(ga4s[pidx]), x4(bea4[pidx])], axis=1)
        cb1 = np.ascontiguousarray(cb1)
        wv = np.asarray(w_val, f32)[sl]
        wg = np.asarray(w_gate, f32)[sl]
        prow = np.concatenate(
            [wv, np.asarray(g_val, f32)[sl],
             np.asarray(be_val, f32)[sl], wg,
             np.asarray(g_gate, f32)[sl],
             np.asarray(be_gate, f32)[sl],
             wv * wv, wg * wg])[None, :]
        cb3 = np.ascontiguousarray(
            np.concatenate([np.ones((1, 128), f32), prow,
                            sba[None, :], sba2[None, :]], axis=1))
        in_maps.append({"a1s": a1s, "cb1": cb1, "cb2": id16, "cb3": cb3,
                        "cbh": idh})
    return in_maps


def _unpack_output(results):
    out = np.empty((N, CA, T, FQ), np.float32)
    for i in range(NCORE):
        r = np.asarray(results[i]["out"]).reshape(B, TV, CL, NS, RP, FQ)
        r = r.transpose(0, 3, 2, 1, 4, 5).reshape(N, CL, T, FQ)
        out[:, i * CL:(i + 1) * CL] = r
    return out


def _install_ntff_shim():
    """The agent image's ``antenv`` lacks ``axon_hooks``; recreate it and
    register the ctypes NTFF hook against /opt/axon/libaxon_pjrt.so (the
    same mechanism trn_boot uses when the module exists)."""
    import sys
    import types
    import ctypes
    import contextlib

    if "antenv.axon_hooks" in sys.modules:
        return True
    so_path = "/opt/axon/libaxon_pjrt.so"
    try:
        lib = ctypes.CDLL(so_path)
    except OSError:
        return False
    if not hasattr(lib, "axon_start_nrt_profile"):
        return False
    lib.axon_start_nrt_profile.argtypes = [ctypes.POINTER(ctypes.c_int64),
                                           ctypes.c_size_t]
    lib.axon_start_nrt_profile.restype = ctypes.c_int64
    lib.axon_stop_nrt_profile.argtypes = [ctypes.c_char_p]
    lib.axon_stop_nrt_profile.restype = ctypes.c_int64

    @contextlib.contextmanager
    def _hook(output_dir, device_ids):
        import jax
        jax.devices()
        if device_ids:
            ids = (ctypes.c_int64 * len(device_ids))(*device_ids)
            rc = lib.axon_start_nrt_profile(ids, len(device_ids))
        else:
            rc = lib.axon_start_nrt_profile(None, 0)
        if rc != 0:
            raise RuntimeError(f"axon_start_nrt_profile rc={rc}")
        try:
            yield
        finally:
            n = lib.axon_stop_nrt_profile(str(output_dir).encode())
            print(f"profile: {n} file(s) written to {output_dir}",
                  file=sys.stderr)

    mod = types.ModuleType("antenv.axon_hooks")
    _state = {"hook": _hook}
    mod.get_axon_ntff_profile_hook = lambda: _state["hook"]

    def set_axon_ntff_profile_hook(h):
        _state["hook"] = h

    mod.set_axon_ntff_profile_hook = set_axon_ntff_profile_hook
    import antenv
    antenv.axon_hooks = mod
    sys.modules["antenv.axon_hooks"] = mod
    return True


def run(inputs, trace=False, **trace_kwargs):
    """Returns (output, BassKernelResults)."""
    nc = _get_nc()
    in_maps = _pack_inputs(**inputs)
    if trace and not _install_ntff_shim():
        trace = False
    br = run_bass_kernel_spmd(nc, in_maps, core_ids=list(range(NCORE)),
                              trace=trace, **trace_kwargs)
    return _unpack_output(br.results), br


def kernel(**inputs):
    out, _ = run(inputs)
    return out
